# revision 1
# baseline (speedup 1.0000x reference)
"""Trainium2 Bass kernel for nn_BlockRF (BatchNorm -> LocallyConnected2D 3x3 valid -> ReLU).

Shapes (hardcoded per the problem spec):
  x:      [B=32, H=64, W=64, C=32]  f32
  gamma/beta/moving_mean/moving_var: [C=32] f32
  weight: [OH*OW=3844, KH*KW*C=288, F=32] f32
  out:    [B=32, OH=62, OW=62, F=32] f32

Strategy: shard over output rows (OH). OH=62 padded to 64 -> 8 rows/core on 8
cores. Each core streams only its slice of the (dominant) weight tensor.

Per core, per output row oh (pipelined via tile pools):
  - x rows r0..r0+2 live in an SBUF tile X[(i,c)=96, (w,b)=2048], BN applied
    in fp16 (VectorE).
  - weights stream per-oh: W[(i,c)=96, (w,g,f)=6144] fp16, where slot (w,g)
    holds the 3x32-channel chunk j=2-g of position ow=w-2+g (the matmul for
    (ow, j) reads slot (w=ow+j, g=2-j)).
  - For each position ow: 3 accumulating matmuls (K=96=3 taps x 32ch,
    M=B=32, N=F=32) into a PSUM slice; lhsT = X w-slice (stationary),
    rhs = weight chunk (moving). fp16 operands, fp32 PSUM accumulation.
  - PSUM tiles hold 32 positions (2 banks); ReLU evacuation (fp16 out)
    alternates between VectorE and ScalarE.
  - Per-oh fp16 output DMA on the scalar HWDGE ring (keeps the sync ring
    exclusively for input streaming); host upcasts to fp32.

Host side only pads/transposes/casts (layout prep + sharding) - all model
arithmetic (BN, conv, ReLU) runs on device.
"""

import numpy as np

B, H, W, C, F = 32, 64, 64, 32, 32
KH = KW = 3
OH = OW = 62
OHP = 64  # padded OH
RPC = OHP // 8  # output rows per core = 8
EPS = 1e-3
NPART = KH * C  # 96 partitions = (i, c)
XFREE = W * B  # 2048
# packed weight slots: for stationary w-slice, valid g values are those with
# ow = w-2+g in [0, OW); slots stored g-ascending, w-major
_GMIN = [max(0, 2 - w) for w in range(W)]
_GMAX = [min(2, OW - 1 - (w - 2)) for w in range(W)]
_SLOT_BASE = [0] * W
for _w in range(1, W):
    _SLOT_BASE[_w] = _SLOT_BASE[_w - 1] + (_GMAX[_w - 1] - _GMIN[_w - 1] + 1)
NSLOT = _SLOT_BASE[-1] + (_GMAX[-1] - _GMIN[-1] + 1)  # 186
WSLOT = NSLOT * F  # 5952
YFREE = OW * F  # 1984
PSUM_POS = 32  # positions per PSUM tile (32*32*4B = 4KB = two banks)

_CACHE = {}


def _build_program():
    import concourse.mybir as mybir
    import concourse.tile as tile
    from concourse import bacc
    from contextlib import ExitStack

    f16 = mybir.dt.float16
    f32 = mybir.dt.float32

    nc = bacc.Bacc("TRN2", target_bir_lowering=False, debug=False, num_devices=8)

    xin = nc.dram_tensor("xin", [RPC, NPART, XFREE], f16, kind="ExternalInput").ap()
    win = nc.dram_tensor("win", [RPC, NPART, WSLOT], f16, kind="ExternalInput").ap()
    pin = nc.dram_tensor("pin", [NPART, 4], f32, kind="ExternalInput").ap()
    yout = nc.dram_tensor("yout", [RPC, B, YFREE], f16, kind="ExternalOutput").ap()

    with ExitStack() as ctx:
        tc = ctx.enter_context(tile.TileContext(nc))
        singles = ctx.enter_context(tc.tile_pool(name="singles", bufs=1))
        xpool = ctx.enter_context(tc.tile_pool(name="xpool", bufs=3))
        xnpool = ctx.enter_context(tc.tile_pool(name="xnpool", bufs=3))
        wpool = ctx.enter_context(tc.tile_pool(name="wpool", bufs=4))
        opool = ctx.enter_context(tc.tile_pool(name="opool", bufs=3))
        pspool = ctx.enter_context(
            tc.tile_pool(name="pspool", bufs=4, space="PSUM")
        )

        # ---- BN affine params: A = gamma/sqrt(var+eps), Bb = beta - mean*A
        par = singles.tile([NPART, 4], f32)
        nc.sync.dma_start(out=par, in_=pin)
        tmp = singles.tile([NPART, 1], f32)
        A = singles.tile([NPART, 1], f32)
        Bb = singles.tile([NPART, 1], f32)
        nc.vector.tensor_scalar_add(tmp, par[:, 3:4], EPS)  # var + eps
        nc.scalar.sqrt(tmp, tmp)
        nc.vector.reciprocal(A, tmp)  # 1/sqrt(var+eps)
        nc.vector.tensor_mul(A, A, par[:, 0:1])  # * gamma
        nc.vector.tensor_mul(tmp, A, par[:, 2:3])  # mean * A
        nc.vector.tensor_sub(Bb, par[:, 1:2], tmp)  # beta - mean*A

        # x loads ride the gpsimd (SWDGE) queue, software-pipelined one oh
        # ahead so each lands before the previous oh's output store can
        # block the queue
        xts = [xpool.tile([NPART, XFREE], f16, name="xt", tag="xt")]
        nc.gpsimd.dma_start(out=xts[0], in_=xin[0])

        HW = WSLOT // 2
        for oh in range(RPC):
            wt = wpool.tile([NPART, WSLOT], f16)
            # split each weight load across both HWDGE rings so the two
            # queues' packets interleave on the SDMA engines (hides the
            # per-transfer doorbell/completion bubble)
            nc.sync.dma_start(out=wt[:, :HW], in_=win[oh][:, :HW])
            nc.scalar.dma_start(out=wt[:, HW:], in_=win[oh][:, HW:])
            if oh + 1 < RPC:
                nxt = xpool.tile([NPART, XFREE], f16, name="xt", tag="xt")
                nc.gpsimd.dma_start(out=nxt, in_=xin[oh + 1])
                xts.append(nxt)
            xt = xts[oh]
            xn = xnpool.tile([NPART, XFREE], f16)
            nc.vector.tensor_scalar(
                xn, xt, A, Bb,
                op0=mybir.AluOpType.mult, op1=mybir.AluOpType.add,
            )

            rowbuf = opool.tile([B, YFREE], f16)
            ngrp = (OW + PSUM_POS - 1) // PSUM_POS
            # PSUM 'start=True' pend-zeroes a whole 2KB bank, so interleaved
            # accumulation slices cannot use it: memset the tile instead and
            # accumulate every matmul (start=False onto zeroed values).
            pstiles = []
            for gi in range(ngrp):
                pst = pspool.tile([B, PSUM_POS * F], mybir.dt.float32,
                                  name="ps", tag="ps")
                nc.vector.memset(pst, 0.0)
                pstiles.append(pst)

            def emit(w, ow_lo, ow_hi):
                # one matmul covering positions ow_lo..ow_hi (inclusive) at
                # stationary w-slice; slots (w, g=2-(w-ow)) are
                # free-contiguous for ascending ow
                grp = ow_lo // PSUM_POS
                s = ow_lo - grp * PSUM_POS
                n = ow_hi - ow_lo + 1
                g_lo = 2 - (w - ow_lo)
                slot = _SLOT_BASE[w] + (g_lo - _GMIN[w])
                nc.tensor.matmul(
                    pstiles[grp][:, s * F:(s + n) * F],
                    xn[:, w * B:(w + 1) * B],
                    wt[:, slot * F:(slot + n) * F],
                    start=False,
                    stop=True,
                    skip_group_check=True,
                )

            BANK_POS = 16  # a matmul out cannot cross a 2KB PSUM bank line
            for w in range(W):
                lo, hi = max(w - 2, 0), min(w, OW - 1)
                if lo > hi:
                    continue
                mid = (lo // BANK_POS) * BANK_POS + BANK_POS - 1
                if hi <= mid:
                    emit(w, lo, hi)
                else:  # straddles a PSUM bank line
                    emit(w, lo, mid)
                    emit(w, mid + 1, hi)

            for grp in range(ngrp):
                npos = min(PSUM_POS, OW - grp * PSUM_POS)
                dst = rowbuf[:, grp * PSUM_POS * F
                             : grp * PSUM_POS * F + npos * F]
                if (oh * ngrp + grp) % 2 == 0:
                    nc.vector.tensor_scalar_max(
                        dst, pstiles[grp][:, : npos * F], 0.0)
                else:
                    nc.scalar.activation(
                        dst, pstiles[grp][:, : npos * F],
                        mybir.ActivationFunctionType.Relu,
                    )
            nc.gpsimd.dma_start(out=yout[oh], in_=rowbuf)

    nc.compile()
    return nc


def _get_program():
    if "nc" not in _CACHE:
        _CACHE["nc"] = _build_program()
    return _CACHE["nc"]


def _prep_inputs(x, gamma, beta, moving_mean, moving_var, weight):
    """Host-side shard/layout/cast prep. Returns per-core in_maps."""
    x = np.asarray(x, dtype=np.float32)
    weight = np.asarray(weight, dtype=np.float32)

    # x: [B,H,W,C] -> pad H to 66 -> transpose to (h, c, w, b), fp16
    xpad = np.zeros((B, H + 2, W, C), np.float32)
    xpad[:, :H] = x
    xt_all = np.ascontiguousarray(xpad.transpose(1, 3, 2, 0)).astype(np.float16)

    # weight: [3844, 288, 32] -> (oh, ow, i, j, c, f) -> (oh, i, c, ow, j, f)
    w6 = weight.reshape(OH, OW, KH, KW, C, F)
    wtr = np.ascontiguousarray(w6.transpose(0, 2, 4, 1, 3, 5)).astype(np.float16)
    # wg[oh, i, c, slot, f]: packed slot (w, g) = position ow=w-2+g, tap j=2-g
    wg = np.zeros((OHP, KH, C, NSLOT, F), np.float16)
    for w in range(W):
        for g in range(_GMIN[w], _GMAX[w] + 1):
            j = 2 - g
            ow = w - 2 + g
            slot = _SLOT_BASE[w] + (g - _GMIN[w])
            wg[:OH, :, :, slot, :] = wtr[:, :, :, ow, j, :]

    p96 = np.tile(
        np.stack([gamma, beta, moving_mean, moving_var], axis=1).astype(np.float32),
        (KH, 1),
    )  # [96, 4]

    in_maps = []
    for k in range(8):
        R = k * RPC
        xc = np.stack(
            [xt_all[R + oh: R + oh + 3].reshape(NPART, XFREE) for oh in range(RPC)]
        )  # [8, 96, 2048]
        wc = np.ascontiguousarray(wg[R: R + RPC]).reshape(RPC, NPART, WSLOT)
        in_maps.append({"xin": xc, "win": wc, "pin": p96})
    return in_maps


def _assemble_output(results):
    """results: list (per core) of {"yout": [RPC, B, YFREE] f16} -> [B,OH,OW,F] f32."""
    yall = np.concatenate([r["yout"] for r in results], axis=0)  # [nrows, B, YFREE]
    y = yall.astype(np.float32).reshape(-1, B, OW, F).transpose(1, 0, 2, 3)
    return np.ascontiguousarray(y[:, :OH] if y.shape[1] >= OH else y)


def run(inputs, trace=False, trace_cores=None):
    """Build/compile/run on 8 cores. Returns (y, BassKernelResults)."""
    from concourse.bass_utils import run_bass_kernel_spmd

    nc = _get_program()
    in_maps = _prep_inputs(**inputs)
    res = run_bass_kernel_spmd(
        nc,
        in_maps,
        core_ids=list(range(8)),
        trace=trace,
        **({"trace_cores": trace_cores} if trace_cores is not None else {}),
    )
    return _assemble_output(res.results), res


def kernel(x, gamma, beta, moving_mean, moving_var, weight):
    y, _ = run(
        dict(x=x, gamma=gamma, beta=beta, moving_mean=moving_mean,
             moving_var=moving_var, weight=weight)
    )
    return y

